# revision 7
# baseline (speedup 1.0000x reference)
"""DeltaSynapse (gnn_message_passing) Trainium2 Bass kernel.

Computes I[b,o] = sum_e signs[e,o]*(W[e,o]*(1-f[e,o]) + Wlong[b,e,o]*f[e,o])
                  * Xpre[b,e,o],
with Xpre[b,e,o] = sum_d delaymap[d,e,o]*Xd[d,b,e]  (one-hot delay gather).

Strategy (8 NeuronCores): shard the postsynaptic axis o into 4 quarters of
512 and the presynaptic axis e into 2 halves of 1024; core (h,q) computes
the partial sum over its e-half for its o-quarter. The two e-half partials
are summed on the host (64KB) and the o-quarters concatenated.

The kernel is HBM-bound (38.25 MiB of f32 reads per core at ~360 GB/s
~= 107 us), so the device schedule keeps the DMA stream saturated:

  - Host hands each core pre-permuted shards so every load is an
    identity-mapped 2D DMA with 8-16 KB contiguous runs per partition.
  - W/STDP/signs for all e-tiles load once up front; A = sgn*W*(1-f) and
    C = sgn*f are derived before the stream needs them. Steady state
    moves only delaymap + Wlong (2 DMAs per 128-e tile, f32->f16 cast in
    the DMA engines).
  - Xd is bit-packed once: packed[e,d] = sum_b 2^b*Xd[d,b,e] via one DMA
    + 2 DVE ops. Per tile the one-hot delay select is a DVE chain
    pi = sum_d packed[:,d]*dmap[d] using per-partition scalar operands
    (exact: pi holds the 8 per-batch spike bits as an integer in fp16).
  - Masks m[b] = (pi>>b)&1 are extracted on DVE/GpSimd, cast to fp16 on
    Scalar/GpSimd, applied in b-pairs t[b] = (A + C*Wlong[b])*m[b]
    (out-of-place fp16 DVE ops run in 2x perf mode), and each pair is
    column-summed into PSUM immediately via a one-hot-column stationary
    matmul so the tail after the last DMA stays short.
"""
import numpy as np
from contextlib import ExitStack

D, B, N = 8, 8, 2048
NO = 512          # o columns per core
NE = 1024         # e rows per core
ET = NE // 128    # e-tiles per core
N_CORES = 8

_NC = None


def _build():
    from concourse import bacc, tile, mybir
    from concourse.alu_op_type import AluOpType as op

    f32 = mybir.dt.float32
    f16 = mybir.dt.float16
    i16 = mybir.dt.int16
    Copy = mybir.ActivationFunctionType.Copy

    nc = bacc.Bacc("TRN2", target_bir_lowering=False, debug=False)

    # Host-permuted layouts (see _in_maps): all loads are identity 2D DMAs.
    dmap_d = nc.dram_tensor("dmap", (NE, D, NO), f32, kind="ExternalInput")
    xd_d = nc.dram_tensor("xd", (128, ET, D, B), f32, kind="ExternalInput")
    wl_d = nc.dram_tensor("wl", (NE, B, NO), f32, kind="ExternalInput")
    w_d = nc.dram_tensor("w", (128, ET, NO), f32, kind="ExternalInput")
    stdp_d = nc.dram_tensor("stdp", (128, ET, NO), f32, kind="ExternalInput")
    sgn_d = nc.dram_tensor("sgn", (128, ET, NO), f32, kind="ExternalInput")
    out_d = nc.dram_tensor("iout", (B, NO), f32, kind="ExternalOutput")

    with tile.TileContext(nc) as tc, ExitStack() as ctx:
        cpool = ctx.enter_context(tc.tile_pool(name="const", bufs=1))
        spool = ctx.enter_context(tc.tile_pool(name="stream", bufs=3))
        wpool = ctx.enter_context(tc.tile_pool(name="work", bufs=2))
        accpool = ctx.enter_context(tc.tile_pool(name="acc", bufs=1, space="PSUM"))

        def load_tile(et):
            # f32->f16 casting DMAs must be initiated from gpsimd
            esl = slice(et * 128, (et + 1) * 128)
            dm3 = spool.tile([128, D, NO], f16, name=f"dm3_{et}", tag="dm3")
            nc.gpsimd.dma_start(dm3[:], dmap_d[esl])
            wl3 = spool.tile([128, B, NO], f16, name=f"wl3_{et}", tag="wl3")
            nc.gpsimd.dma_start(wl3[:], wl_d[esl])
            return dm3, wl3

        # ---- streaming loads first: tile 0 leads, aux tensors go on the
        # scalar engine's queue so they run in parallel with the stream.
        pre = {0: load_tile(0)}
        xd_sb = cpool.tile([128, ET, D, B], f32)
        nc.scalar.dma_start(xd_sb[:], xd_d[:])
        w_sb = cpool.tile([128, ET, NO], f16)
        nc.gpsimd.dma_start(w_sb[:], w_d[:])
        stdp_sb = cpool.tile([128, ET, NO], f16)
        nc.gpsimd.dma_start(stdp_sb[:], stdp_d[:])
        sgn_sb = cpool.tile([128, ET, NO], f16)
        nc.gpsimd.dma_start(sgn_sb[:], sgn_d[:])
        for et in (1, 2):
            pre[et] = load_tile(et)

        # ---- constants ------------------------------------------------
        ebs = []
        for b in range(B):
            ebt = cpool.tile([128, B], f16, name=f"eb{b}")
            nc.vector.memset(ebt[:], 0.0)
            nc.vector.memset(ebt[:, b:b + 1], 1.0)
            ebs.append(ebt)
        pw = cpool.tile([128, 1, 1, B], f32)
        for b in range(B):
            nc.vector.memset(pw[:, :, :, b], float(1 << b))

        # ---- pack Xd: packed16[e, et, d] = sum_b 2^b * Xd[d, b, e] ----
        xw = cpool.tile([128, ET, D, B], f32)
        nc.vector.tensor_tensor(
            xw[:], xd_sb[:], pw[:].broadcast_to((128, ET, D, B)), op=op.mult)
        # packed stays f32: tensor_scalar/stt per-partition scalars must be f32
        packed16 = cpool.tile([128, ET, D], f32)
        nc.vector.tensor_reduce(
            packed16[:], xw[:], axis=mybir.AxisListType.X, op=op.add)

        # ---- A = sgn*W*(1-f), C = sgn*f for all tiles (fp16) ----------
        omf = cpool.tile([128, ET, NO], f16)
        nc.scalar.activation(omf[:], stdp_sb[:], Copy, bias=1.0, scale=-1.0)
        C_sb = cpool.tile([128, ET, NO], f16)
        nc.vector.tensor_tensor(C_sb[:], sgn_sb[:], stdp_sb[:], op=op.mult)
        # reuse stdp_sb for sgn*W and w_sb for A (their data is consumed)
        nc.vector.tensor_tensor(stdp_sb[:], sgn_sb[:], w_sb[:], op=op.mult)
        A_sb = cpool.tile([128, ET, NO], f16)
        nc.vector.tensor_tensor(A_sb[:], stdp_sb[:], omf[:], op=op.mult)

        acc = accpool.tile([B, NO], f32)

        # ---- main loop over e-tiles -----------------------------------
        for et in range(ET):
            dm3, wl3 = pre.pop(et) if et in pre else load_tile(et)
            if et + 3 < ET:
                pre[et + 3] = load_tile(et + 3)

            # pi[e,o] = sum_d packed[e,d] * dmap[d,e,o]  (DVE chain with
            # per-partition scalar operand; exact integers in fp16)
            pf = wpool.tile([128, NO], f16, tag="pf")
            nc.vector.tensor_scalar(
                pf[:], dm3[:, 0, :], packed16[:, et, 0:1], None, op0=op.mult)
            for d in range(1, D):
                nc.vector.scalar_tensor_tensor(
                    pf[:], dm3[:, d, :], packed16[:, et, d:d + 1], pf[:],
                    op0=op.mult, op1=op.add)
            pi_i = wpool.tile([128, NO], i16, tag="pi")
            nc.vector.tensor_copy(pi_i[:], pf[:])

            # masks m[b] = (pi >> b) & 1; split engines to keep DVE light
            m_i = wpool.tile([128, B, NO], i16, tag="m_i")
            for b in range(B):
                nc.vector.tensor_scalar(
                    m_i[:, b, :], pi_i[:], b, 1,
                    op0=op.logical_shift_right, op1=op.bitwise_and)
            m_f = wpool.tile([128, B, NO], f16, tag="m_f")
            for k in range(3):
                nc.scalar.activation(
                    m_f[:, 2 * k:2 * k + 2, :], m_i[:, 2 * k:2 * k + 2, :],
                    Copy)
            nc.gpsimd.tensor_copy(m_f[:, 6:8, :], m_i[:, 6:8, :])

            # v = A + C*Wlong (out-of-place fp16 -> DVE 2x mode)
            u = wpool.tile([128, B, NO], f16, tag="u")
            nc.vector.tensor_tensor(
                u[:], wl3[:],
                C_sb[:, et, :].unsqueeze(1).broadcast_to((128, B, NO)),
                op=op.mult)
            v = wpool.tile([128, B, NO], f16, tag="v")
            nc.vector.tensor_tensor(
                v[:], u[:],
                A_sb[:, et, :].unsqueeze(1).broadcast_to((128, B, NO)),
                op=op.add)

            # t[b] = v[b]*m[b] in b-pairs; column-sum each pair into PSUM
            # immediately (keeps the last tile's tail short)
            t = wpool.tile([128, B, NO], f16, tag="t")
            for k in range(4):
                bsl = slice(2 * k, 2 * k + 2)
                nc.vector.tensor_tensor(
                    t[:, bsl, :], v[:, bsl, :], m_f[:, bsl, :], op=op.mult)
                for b in (2 * k, 2 * k + 1):
                    nc.tensor.matmul(
                        acc[:], ebs[b][:], t[:, b, :],
                        start=(et == 0 and b == 0),
                        stop=(et == ET - 1 and b == B - 1))

        out_sb = cpool.tile([B, NO], f32)
        nc.vector.tensor_copy(out_sb[:], acc[:])
        nc.sync.dma_start(out_d[:], out_sb[:])

    nc.compile()
    return nc


def _in_maps(Xd, delaymap, W, Wlong, STDP_frac, signs):
    def emat(x):  # (NE, NO) slice -> (128, ET, NO): partition-major rows
        return np.ascontiguousarray(
            x.reshape(ET, 128, NO).transpose(1, 0, 2))

    maps = []
    for c in range(N_CORES):
        h, q = divmod(c, 4)
        e0, o0 = h * NE, q * NO
        es, os_ = slice(e0, e0 + NE), slice(o0, o0 + NO)
        xd_c = Xd[:, :, es].transpose(2, 0, 1)          # (NE, D, B)
        maps.append({
            "dmap": np.ascontiguousarray(
                delaymap[:, es, os_].transpose(1, 0, 2)),   # (NE, D, NO)
            "xd": np.ascontiguousarray(
                xd_c.reshape(ET, 128, D, B).transpose(1, 0, 2, 3)),
            "wl": np.ascontiguousarray(
                Wlong[:, es, os_].transpose(1, 0, 2)),      # (NE, B, NO)
            "w": emat(W[es, os_]),
            "stdp": emat(STDP_frac[es, os_]),
            "sgn": emat(signs[es, os_]),
        })
    return maps


def _gather(outs):
    return np.concatenate(
        [outs[q] + outs[q + 4] for q in range(4)], axis=1).astype(np.float32)


def kernel(Xd, delaymap, W, Wlong, STDP_frac, signs):
    global _NC
    from concourse.bass_utils import run_bass_kernel_spmd
    if _NC is None:
        _NC = _build()
    maps = _in_maps(Xd, delaymap, W, Wlong, STDP_frac, signs)
    res = run_bass_kernel_spmd(_NC, maps, list(range(N_CORES)))
    return _gather([r["iout"] for r in res.results])


# revision 9
# speedup vs baseline: 1.2110x; 1.2110x over previous
"""DeltaSynapse (gnn_message_passing) Trainium2 Bass kernel.

Computes I[b,o] = sum_e signs[e,o]*(W[e,o]*(1-f[e,o]) + Wlong[b,e,o]*f[e,o])
                  * Xpre[b,e,o],
with Xpre[b,e,o] = sum_d delaymap[d,e,o]*Xd[d,b,e]  (one-hot delay gather).

Strategy (8 NeuronCores): shard the postsynaptic axis o into 4 quarters of
512 and the presynaptic axis e into 2 halves of 1024; core (h,q) computes
the partial sum over its e-half for its o-quarter. The two e-half partials
are summed on the host (64KB) and the o-quarters concatenated.

The kernel is HBM-bound (38.25 MiB of f32 reads per core at ~380 GB/s
~= 105 us), so the schedule keeps the DMA stream saturated end to end:

  - Host hands each core pre-permuted shards so every load is an
    identity-mapped DMA with 8-16 KB contiguous runs per partition.
  - Aux tensors (Xd, W, STDP, signs) are issued FIRST; engines' static
    program order then computes packed spikes and A = sgn*W*(1-f),
    C = sgn*f before the first stream tile's elementwise work needs
    them, and the DMA queue is never idle behind compute.
  - Steady state streams delaymap (d-split halves) + Wlong per 128-e
    tile, cast f32->f16 in the DMA engines (gpsimd-issued SWDGE).
  - Xd is bit-packed once (packed[e,d] = sum_b 2^b*Xd[d,b,e]); per tile
    the one-hot delay select runs on the PE as sum_d diag(packed[:,d])
    @ dmap[d], landing all 8 per-batch spike bits as an exact integer
    Pi in PSUM. Masks m[b] = (Pi>>b)&1 extract on DVE in b-pairs, cast
    i16->f16 on Scalar, and t[b] = (A + C*Wlong[b])*m[b] applies on DVE
    with out-of-place fp16 ops (2x perf mode); one-hot-column matmuls
    column-sum t into PSUM across all tiles.
  - The last e-tile is stored o-half-major so it streams and computes
    as two independent 256-column half-tiles: the post-stream tail is
    one short half-tile chain instead of a full-tile one.
"""
import numpy as np
from contextlib import ExitStack

D, B, N = 8, 8, 2048
NO = 512          # o columns per core
NE = 1024         # e rows per core
ET = NE // 128    # e-tiles per core
LT = ET - 1       # last tile (o-split)
N_CORES = 8

_NC = None


def _build():
    from concourse import bacc, tile, mybir, masks
    from concourse.alu_op_type import AluOpType as op

    f32 = mybir.dt.float32
    f16 = mybir.dt.float16
    i16 = mybir.dt.int16
    Copy = mybir.ActivationFunctionType.Copy

    nc = bacc.Bacc("TRN2", target_bir_lowering=False, debug=False)

    # Host-permuted layouts (see _in_maps): all loads are identity DMAs.
    dmap_d = nc.dram_tensor("dmap", (NE - 128, D, NO), f32, kind="ExternalInput")
    dmap7_d = nc.dram_tensor("dmap7", (128, 2, D, NO // 2), f32, kind="ExternalInput")
    xd_d = nc.dram_tensor("xd", (128, ET, D, B), f32, kind="ExternalInput")
    wl_d = nc.dram_tensor("wl", (NE - 128, B, NO), f32, kind="ExternalInput")
    wl7_d = nc.dram_tensor("wl7", (128, 2, B, NO // 2), f32, kind="ExternalInput")
    w_d = nc.dram_tensor("w", (128, ET, NO), f32, kind="ExternalInput")
    stdp_d = nc.dram_tensor("stdp", (128, ET, NO), f32, kind="ExternalInput")
    sgn_d = nc.dram_tensor("sgn", (128, ET, NO), f32, kind="ExternalInput")
    out_d = nc.dram_tensor("iout", (B, NO), f32, kind="ExternalOutput")

    with tile.TileContext(nc) as tc, ExitStack() as ctx:
        cpool = ctx.enter_context(tc.tile_pool(name="const", bufs=1))
        spool = ctx.enter_context(tc.tile_pool(name="stream", bufs=3))
        wpool = ctx.enter_context(tc.tile_pool(name="work", bufs=2))
        mpool = ctx.enter_context(tc.tile_pool(name="mpair", bufs=4))
        pspool = ctx.enter_context(tc.tile_pool(name="pst", bufs=2, space="PSUM"))
        accpool = ctx.enter_context(tc.tile_pool(name="acc", bufs=1, space="PSUM"))

        # ---- aux tensors first: the whole stream stays FIFO-busy and
        # A/C are ready before tile 0's elementwise work needs them.
        xd_sb = cpool.tile([128, ET, D, B], f32)
        nc.scalar.dma_start(xd_sb[:], xd_d[:])
        w_sb = cpool.tile([128, ET, NO], f16)
        nc.gpsimd.dma_start(w_sb[:], w_d[:])
        stdp_sb = cpool.tile([128, ET, NO], f16)
        nc.gpsimd.dma_start(stdp_sb[:], stdp_d[:])
        sgn_sb = cpool.tile([128, ET, NO], f16)
        nc.gpsimd.dma_start(sgn_sb[:], sgn_d[:])

        def load_tile(et):
            # f32->f16 casting DMAs must initiate from gpsimd (SWDGE).
            # dmap comes in d-halves so the PE select can start early.
            esl = slice(et * 128, (et + 1) * 128)
            dma_ = spool.tile([128, 4, NO], f16, name=f"dma_{et}", tag="dma")
            nc.gpsimd.dma_start(dma_[:], dmap_d[esl, 0:4])
            dmb = spool.tile([128, 4, NO], f16, name=f"dmb_{et}", tag="dmb")
            nc.gpsimd.dma_start(dmb[:], dmap_d[esl, 4:8])
            wl3 = spool.tile([128, B, NO], f16, name=f"wl3_{et}", tag="wl3")
            nc.gpsimd.dma_start(wl3[:], wl_d[esl])
            return dma_, dmb, wl3

        def load_tile7():
            halves = []
            for oh in range(2):
                dm7 = cpool.tile([128, D, NO // 2], f16, name=f"dm7_{oh}")
                nc.gpsimd.dma_start(dm7[:], dmap7_d[:, oh])
                wl7 = cpool.tile([128, B, NO // 2], f16, name=f"wl7_{oh}")
                nc.gpsimd.dma_start(wl7[:], wl7_d[:, oh])
                halves.append((dm7, wl7))
            return halves

        pre = {et: load_tile(et) for et in (0, 1, 2)}

        # ---- constants ------------------------------------------------
        ebs = []
        for b in range(B):
            ebt = cpool.tile([128, B], f16, name=f"eb{b}")
            nc.vector.memset(ebt[:], 0.0)
            nc.vector.memset(ebt[:, b:b + 1], 1.0)
            ebs.append(ebt)
        pw = cpool.tile([128, 1, 1, B], f32)
        for b in range(B):
            nc.vector.memset(pw[:, :, :, b], float(1 << b))
        ident3 = cpool.tile([128, D, 128], f16)
        for d in range(D):
            masks.make_identity(nc, ident3[:, d, :])

        # ---- pack Xd: packed16[e, et, d] = sum_b 2^b * Xd[d, b, e] ----
        xw = cpool.tile([128, ET, D, B], f32)
        nc.vector.tensor_tensor(
            xw[:], xd_sb[:], pw[:].broadcast_to((128, ET, D, B)), op=op.mult)
        packed = cpool.tile([128, ET, D], f32)
        nc.vector.tensor_reduce(
            packed[:], xw[:], axis=mybir.AxisListType.X, op=op.add)
        packed16 = cpool.tile([128, ET, D], f16)
        nc.vector.tensor_copy(packed16[:], packed[:])

        # ---- A = sgn*W*(1-f), C = sgn*f for all tiles (fp16) ----------
        omf = cpool.tile([128, ET, NO], f16)
        nc.scalar.activation(omf[:], stdp_sb[:], Copy, bias=1.0, scale=-1.0)
        C_sb = cpool.tile([128, ET, NO], f16)
        nc.vector.tensor_tensor(C_sb[:], sgn_sb[:], stdp_sb[:], op=op.mult)
        # reuse stdp_sb for sgn*W and A_sb out-of-place (data consumed)
        nc.vector.tensor_tensor(stdp_sb[:], sgn_sb[:], w_sb[:], op=op.mult)
        A_sb = cpool.tile([128, ET, NO], f16)
        nc.vector.tensor_tensor(A_sb[:], stdp_sb[:], omf[:], op=op.mult)

        acc = accpool.tile([B, NO], f32)

        def emit(et, osl, dms, wlv, first, last):
            """Mask chain + blend + column-sum for o-slice osl of tile et.
            dms: [128, nd, now] fp16 delaymap views covering d=0..7 in
            order; wlv: [128, B, now]."""
            now = osl.stop - osl.start
            dstack = wpool.tile([128, D, 128], f16, tag="dstack")
            nc.vector.tensor_tensor(
                dstack[:], ident3[:],
                packed16[:, et, :].unsqueeze(-1).broadcast_to((128, D, 128)),
                op=op.mult)
            pi_ps = pspool.tile([128, NO], f32, tag="pi_ps")
            piv = pi_ps[:, 0:now]
            d = 0
            for dmv in dms:
                for j in range(dmv.shape[1]):
                    nc.tensor.matmul(
                        piv, dstack[:, d, :], dmv[:, j, :],
                        start=(d == 0), stop=(d == D - 1))
                    d += 1
            pi_i = wpool.tile([128, NO], i16, tag="pi")
            piiv = pi_i[:, 0:now]
            nc.vector.tensor_copy(piiv, piv)

            # u = C*Wl, v = u + A (out-of-place fp16)
            u = wpool.tile([128, B, NO], f16, tag="u")
            uv = u[:, :, 0:now]
            nc.vector.tensor_tensor(
                uv, wlv,
                C_sb[:, et, osl].unsqueeze(1).broadcast_to((128, B, now)),
                op=op.mult)
            v = wpool.tile([128, B, NO], f16, tag="v")
            vv = v[:, :, 0:now]
            nc.vector.tensor_tensor(
                vv, uv,
                A_sb[:, et, osl].unsqueeze(1).broadcast_to((128, B, now)),
                op=op.add)

            # masks in b-pairs: DVE shifts feed i16->f16 casts on Scalar
            m_f = wpool.tile([128, B, NO], f16, tag="m_f")
            for k in range(4):
                m_i = mpool.tile([128, 2, NO], i16, tag="m_i")
                for j in range(2):
                    nc.vector.tensor_scalar(
                        m_i[:, j, 0:now], piiv, 2 * k + j, 1,
                        op0=op.logical_shift_right, op1=op.bitwise_and)
                nc.scalar.activation(
                    m_f[:, 2 * k:2 * k + 2, 0:now], m_i[:, :, 0:now], Copy)

            # t = v*m reuses u's buffer (u is dead after v)
            nc.vector.tensor_tensor(uv, vv, m_f[:, :, 0:now], op=op.mult)
            for b in range(B):
                nc.tensor.matmul(
                    acc[:, osl], ebs[b][:], u[:, b, 0:now],
                    start=(first and b == 0), stop=(last and b == B - 1),
                    skip_group_check=True)

        # ---- main loop ------------------------------------------------
        for et in range(LT):
            dma_, dmb, wl3 = pre.pop(et)
            if et + 3 < LT:
                pre[et + 3] = load_tile(et + 3)
            elif et + 3 == LT:
                pre[LT] = load_tile7()
            emit(et, slice(0, NO), [dma_[:], dmb[:]], wl3[:],
                 first=(et == 0), last=False)

        halves = pre.pop(LT)
        for oh in range(2):
            dm7, wl7 = halves[oh]
            emit(LT, slice(oh * (NO // 2), (oh + 1) * (NO // 2)),
                 [dm7[:]], wl7[:], first=False, last=(oh == 1))

        out_sb = cpool.tile([B, NO], f32)
        nc.vector.tensor_copy(out_sb[:], acc[:])
        nc.sync.dma_start(out_d[:], out_sb[:])

    nc.compile()
    return nc


def _in_maps(Xd, delaymap, W, Wlong, STDP_frac, signs):
    def emat(x):  # (NE, NO) slice -> (128, ET, NO): partition-major rows
        return np.ascontiguousarray(
            x.reshape(ET, 128, NO).transpose(1, 0, 2))

    maps = []
    for c in range(N_CORES):
        h, q = divmod(c, 4)
        e0, o0 = h * NE, q * NO
        es, os_ = slice(e0, e0 + NE), slice(o0, o0 + NO)
        e7 = slice(e0 + NE - 128, e0 + NE)
        xd_c = Xd[:, :, es].transpose(2, 0, 1)          # (NE, D, B)
        dm = delaymap[:, es, os_].transpose(1, 0, 2)    # (NE, D, NO)
        wl = Wlong[:, es, os_].transpose(1, 0, 2)       # (NE, B, NO)
        # last 128-e tile: o-half-major so it streams as 2 half-tiles
        dm7 = delaymap[:, e7, os_].transpose(1, 0, 2).reshape(
            128, D, 2, NO // 2).transpose(0, 2, 1, 3)   # (128, 2, D, 256)
        wl7 = Wlong[:, e7, os_].transpose(1, 0, 2).reshape(
            128, B, 2, NO // 2).transpose(0, 2, 1, 3)   # (128, 2, B, 256)
        maps.append({
            "dmap": np.ascontiguousarray(dm[:NE - 128]),
            "dmap7": np.ascontiguousarray(dm7),
            "xd": np.ascontiguousarray(
                xd_c.reshape(ET, 128, D, B).transpose(1, 0, 2, 3)),
            "wl": np.ascontiguousarray(wl[:NE - 128]),
            "wl7": np.ascontiguousarray(wl7),
            "w": emat(W[es, os_]),
            "stdp": emat(STDP_frac[es, os_]),
            "sgn": emat(signs[es, os_]),
        })
    return maps


def _gather(outs):
    return np.concatenate(
        [outs[q] + outs[q + 4] for q in range(4)], axis=1).astype(np.float32)


def kernel(Xd, delaymap, W, Wlong, STDP_frac, signs):
    global _NC
    from concourse.bass_utils import run_bass_kernel_spmd
    if _NC is None:
        _NC = _build()
    maps = _in_maps(Xd, delaymap, W, Wlong, STDP_frac, signs)
    res = run_bass_kernel_spmd(_NC, maps, list(range(N_CORES)))
    return _gather([r["iout"] for r in res.results])
